# revision 2
# baseline (speedup 1.0000x reference)
"""Distributed 2-layer GCN for 8 Trainium2 NeuronCores — v3.

Strategy (matches the sharding hint):
- Destination nodes are sharded across the 8 cores (stripe-interleaved so that
  chunked AllGathers land node-contiguous); edges are partitioned by destination
  so scatter-add aggregation is core-local.
- Layer 1 aggregates over the raw (replicated) X first (matmul commutes with the
  normalized aggregation), so no collective is needed for layer 1.
- Each core then applies W1+relu+W2 to its own shard; the small [*, 64]
  post-W2 activations are all-gathered in 4 node-contiguous chunks
  (pipelined against layer-1 compute) into a single [NPAD, 64] layer-2 table.
- Aggregation: edges sorted by (core, 128-dst-block, src-window), padded into
  128-edge tiles. Per tile an indexed DMA gathers the 128 source rows onto the
  128 SBUF partitions and the tensor engine computes g^T @ S into a PSUM bank
  holding 512 destination slots, where S[e, d] = norm_e * [dst_e == d].

v3 perf notes:
- dma_gather descriptor generation on the Q7 cores is the critical path;
  gathers round-robin all 4 SWDGE queues so all 4 Q7 pairs generate
  descriptors concurrently.
- X is pre-rounded to bf16 host-side: L1 gathers move half the bytes and
  feed the matmul directly.
- S tiles are precomputed host-side (pure edge metadata: norm x onehot(dst))
  and streamed as sequential HWDGE DMAs, which keeps the vector engine off
  the critical path entirely.
- Self-loop edges are dropped from the layer-2 edge stream; their (diagonal)
  contribution is computed from the SBUF-resident h2 tiles of the owning
  core via an identity matmul, saving ~5% of gather descriptors.
- Source windows are 32768 rows (the int16 index limit) rather than 25088,
  which packs edge tiles slightly fuller.
"""

import numpy as np

# problem shape (hardcoded per the task contract)
N = 100000
E = 1600000
F1 = 128
F2 = 64
CORES = 8
STRIPE = 3136          # owned rows per (core, stripe)
SH = 4 * STRIPE        # owned rows per core
WROW = 8 * STRIPE      # rows per AllGather chunk
NPAD = 32 * STRIPE     # padded node count
NBLK = -(-SH // 128)   # 128-node blocks per core
NBG = -(-NBLK // 4)    # PSUM bankgroups per core
NAG = 4                # AllGather chunks
# gather window bounds: layer 1 reads X (32768-row windows, the int16 index
# limit); layer 2 reads the AllGather chunk tensors (windows must align)
WB1 = [0, 32768, 65536, 98304, NPAD]
WB2 = [0, WROW, 2 * WROW, 3 * WROW, NPAD]
NW = 4
L2SELF_DENSE = True   # False: self loops stay in the L2 edge stream


def _bg_blocks(bg):
    return range(4 * bg, min(4 * bg + 4, NBLK))


def _prep_layer(src, dst, norm, WB):
    """Tile tables for one layer's edge set. Returns per-core packed
    idx [CORES, 128, TOT*8], S tables [CORES, 128, TOT*128] (f32, cast
    later), and the (ranges, tmax, TOT) structure shared by all cores."""
    q = dst // WROW
    v = dst % WROW
    core = v // STRIPE
    owned = q * STRIPE + (v % STRIPE)
    block = owned // 128
    win = np.searchsorted(WB, src, side="right") - 1
    dst_rel = owned % 128

    key = (core * NBLK + block) * NW + win
    counts = np.bincount(key, minlength=CORES * NBLK * NW).reshape(CORES, NBLK, NW)
    tmax = -(-counts.max(axis=0) // 128)           # [NBLK, NW]

    jt0 = np.zeros((NBLK, NW), dtype=np.int64)
    tot = 0
    ranges = []                                     # [bg][w] -> (jt0, jt1)
    for bg in range(NBG):
        per_w = []
        for wn in range(NW):
            start = tot
            for b in _bg_blocks(bg):
                jt0[b, wn] = tot
                tot += tmax[b, wn]
            per_w.append((start, tot))
        ranges.append(per_w)

    order = np.lexsort((src, win, block, core))
    s_src, s_win, s_core, s_block = src[order], win[order], core[order], block[order]
    s_norm, s_dstrel = norm[order], dst_rel[order]

    run_key = (s_core * NBLK + s_block) * NW + s_win
    run_starts = np.flatnonzero(np.r_[True, run_key[1:] != run_key[:-1]])
    run_lens = np.diff(np.r_[run_starts, len(run_key)])
    within = np.arange(len(run_key)) - np.repeat(run_starts, run_lens)
    slot = jt0[s_block, s_win] * 128 + within      # edge slot within core
    wbase = np.asarray(WB, dtype=np.int64)[s_win]

    idx16 = np.zeros((CORES, tot * 128), dtype=np.int16)
    pos = s_core * (tot * 128) + slot
    idx16.reshape(-1)[pos] = (s_src - wbase).astype(np.int16)

    # packed idx: [CORES, 128, TOT*8] — wrapped in 16 partitions, replicated
    # 8x so any Q7 pair finds its slice
    packed = np.zeros((CORES, 128, tot * 8), dtype=np.int16)
    seg = idx16.reshape(CORES, tot * 8, 16)
    packed[:] = np.tile(seg.transpose(0, 2, 1), (1, 8, 1))

    # S tables: [CORES, 128 partitions(edge%128), TOT*128 (tile*128 + dstcol)]
    import ml_dtypes
    stab = np.zeros((CORES, 128, tot * 128), dtype=ml_dtypes.bfloat16)
    p = slot % 128
    colbase = (slot // 128) * 128
    stab[s_core, p, colbase + s_dstrel] = s_norm.astype(ml_dtypes.bfloat16)

    return packed, stab, {"ranges": ranges, "tmax": tmax, "tottiles": tot}


def _prep(edge_index, edge_weights):
    row = np.asarray(edge_index[0], dtype=np.int64)
    col = np.asarray(edge_index[1], dtype=np.int64)
    w = np.asarray(edge_weights, dtype=np.float32)

    deg = np.bincount(col, weights=w.astype(np.float64), minlength=N).astype(np.float32) + 1.0
    dis = (1.0 / np.sqrt(deg)).astype(np.float32)

    # layer 1: graph edges + explicit self loops (weight 1/deg)
    self_ids = np.arange(NPAD, dtype=np.int64)
    self_norm = np.zeros(NPAD, dtype=np.float32)
    self_norm[:N] = 1.0 / deg
    src1 = np.concatenate([row, self_ids])
    dst1 = np.concatenate([col, self_ids])
    norm1 = np.concatenate([(dis[row] * w * dis[col]).astype(np.float32), self_norm])
    idx1, s1, st1 = _prep_layer(src1, dst1, norm1, WB1)

    # layer 2: graph edges only; self loops are applied as a dense diagonal
    # from the SBUF-resident h2 tiles
    if L2SELF_DENSE:
        idx2, s2, st2 = _prep_layer(row, col, (dis[row] * w * dis[col]).astype(np.float32), WB2)
    else:
        idx2, s2, st2 = _prep_layer(src1, dst1, norm1, WB2)

    # per-core self-norm, laid out [128 (dst-in-block), NBLK]
    o2g = _owned_to_global()                        # [CORES, SH]
    sn = self_norm[o2g]                             # [CORES, SH]
    snorm = np.ascontiguousarray(
        sn.reshape(CORES, NBLK, 128).transpose(0, 2, 1)).astype(np.float32)

    return idx1, s1, st1, idx2, s2, st2, snorm


def _owned_to_global():
    r = np.arange(SH)
    q = r // STRIPE
    u = r % STRIPE
    c = np.arange(CORES)[:, None]
    return WROW * q[None, :] + STRIPE * c + u[None, :]     # [CORES, SH]


def _build_program(st1, st2):
    import concourse.bacc as bacc
    import concourse.mybir as mybir
    import concourse.tile as tile

    f32 = mybir.dt.float32
    bf16 = mybir.dt.bfloat16
    i16 = mybir.dt.int16
    Alu = mybir.AluOpType
    Act = mybir.ActivationFunctionType

    TOT1 = st1["tottiles"]
    TOT2 = st2["tottiles"]

    nc = bacc.Bacc("TRN2", target_bir_lowering=False, debug=False,
                   num_devices=CORES, num_swdge_queues=4)
    X = nc.dram_tensor("x", [NPAD, F1], bf16, kind="ExternalInput")
    IDX1 = nc.dram_tensor("idx1", [128, TOT1 * 8], i16, kind="ExternalInput")
    SIN1 = nc.dram_tensor("sin1", [128, TOT1 * 128], bf16, kind="ExternalInput")
    IDX2 = nc.dram_tensor("idx2", [128, TOT2 * 8], i16, kind="ExternalInput")
    SIN2 = nc.dram_tensor("sin2", [128, TOT2 * 128], bf16, kind="ExternalInput")
    SNORM = nc.dram_tensor("snorm", [128, NBLK], f32, kind="ExternalInput")
    IDENT = nc.dram_tensor("ident", [128, 128], bf16, kind="ExternalInput")
    W1 = nc.dram_tensor("w1", [F1, F1], f32, kind="ExternalInput")
    B1 = nc.dram_tensor("b1", [F1, 1], f32, kind="ExternalInput")
    W2 = nc.dram_tensor("w2", [F1, F2], f32, kind="ExternalInput")
    B2 = nc.dram_tensor("b2", [F2, 1], f32, kind="ExternalInput")
    OUT = nc.dram_tensor("out", [F2, SH], f32, kind="ExternalOutput")

    def last_tile_of_group(groups):
        last = None
        for wn, blk, tcount in groups:
            if tcount > 0:
                last = (wn, blk, tcount - 1)
        return last

    with tile.TileContext(nc) as tc:
        with (
            tc.tile_pool(name="const", bufs=1) as cpool,
            tc.tile_pool(name="gth", bufs=5) as gpool,
            tc.tile_pool(name="sb", bufs=5) as spool,
            tc.tile_pool(name="idxp", bufs=12) as ipool,
            tc.tile_pool(name="acc", bufs=2) as apool,
            tc.tile_pool(name="dram", bufs=1, space="DRAM") as dpool,
            tc.tile_pool(name="pagg", bufs=2, space="PSUM") as pagg,
            tc.tile_pool(name="pdense", bufs=2, space="PSUM") as pdense,
            tc.tile_pool(name="pw2", bufs=2, space="PSUM") as pw2,
            tc.tile_pool(name="pagg2", bufs=2, space="PSUM") as pagg2,
        ):
            ag_in = [dpool.tile([STRIPE, F2], f32, tag=f"agin{j}", name=f"agin{j}")
                     for j in range(NAG)]
            out_w = [dpool.tile([WROW, F2], f32, tag=f"agout{j}", name=f"agout{j}",
                                addr_space="Shared")
                     for j in range(NAG)]
            w1t = cpool.tile([F1, F1], f32)
            w2t = cpool.tile([F1, F2], f32)
            b1t = cpool.tile([F1, 1], f32)
            b2t = cpool.tile([F2, 1], f32)
            snormt = cpool.tile([128, NBLK], f32)
            identt = cpool.tile([128, 128], bf16)
            acc2 = cpool.tile([F2, SH], f32)
            h2bf = cpool.tile([128, NBLK, F2], bf16)
            zl1 = cpool.tile([1, 128], bf16)
            zl2 = cpool.tile([1, F2], bf16)
            zr = cpool.tile([1, 512], bf16)
            for t_, d_ in [(w1t, W1), (w2t, W2), (b1t, B1), (b2t, B2),
                           (snormt, SNORM), (identt, IDENT)]:
                nc.sync.dma_start(out=t_[:], in_=d_[:])
            nc.vector.memset(zl1[:], 0.0)
            nc.vector.memset(zl2[:], 0.0)
            nc.vector.memset(zr[:], 0.0)

            def agg_group(st, IDX, SIN, psum, feat, table_ap, bg, wn,
                          last_info, queue, fp32_src, gtag):
                """Aggregate (bg, wn) tiles into psum."""
                a, b = st["ranges"][bg][wn]
                if a == b:
                    return False
                T = b - a
                idxt = ipool.tile([128, T * 8], i16, tag="idx")
                nc.sync.dma_start(out=idxt[:], in_=IDX[:, a * 8:b * 8])
                if fp32_src:
                    graw = gpool.tile([128, T, feat], f32, tag=gtag + "raw")
                    nc.gpsimd.dma_gather(
                        out_ap=graw[:], in_ap=table_ap, idxs_ap=idxt[:],
                        num_idxs=T * 128, num_idxs_reg=T * 128, elem_size=feat,
                        single_packet=False, queue_num=queue,
                    )
                    g = gpool.tile([128, T, feat], bf16, tag=gtag)
                    nc.scalar.activation(out=g[:], in_=graw[:], func=Act.Copy)
                else:
                    g = gpool.tile([128, T, feat], bf16, tag=gtag)
                    nc.gpsimd.dma_gather(
                        out_ap=g[:], in_ap=table_ap, idxs_ap=idxt[:],
                        num_idxs=T * 128, num_idxs_reg=T * 128, elem_size=feat,
                        single_packet=False, queue_num=queue,
                    )
                S = spool.tile([128, T, 128], bf16, tag="S")
                nc.sync.dma_start(out=S[:], in_=SIN[:, a * 128:b * 128])
                jt = a
                for blk in _bg_blocks(bg):
                    tcount = st["tmax"][blk, wn]
                    off = (blk - 4 * bg) * 128
                    for t in range(tcount):
                        ti = jt - a + t
                        is_last = last_info == (wn, blk, t)
                        nc.tensor.matmul(out=psum[:, off:off + 128], lhsT=g[:, ti, :],
                                         rhs=S[:, ti, :], start=False, stop=is_last)
                    jt += tcount
                return True

            # ---------------- layer 1 ----------------
            # fire AllGather j once ag_in[j] (owned rows [j*STRIPE,(j+1)*STRIPE))
            # is fully written, i.e. after bankgroup ceil(STRIPE*(j+1)/512)-1
            ag_fire = {}
            for j in range(NAG):
                ag_fire.setdefault(-(-STRIPE * (j + 1) // 512) - 1, []).append(j)
            r1, t1 = st1["ranges"], st1["tmax"]
            for bg in range(NBG):
                nch = len(list(_bg_blocks(bg)))
                ps = pagg.tile([128, 512], f32, tag="aggps")
                nc.tensor.matmul(out=ps[:], lhsT=zl1[:], rhs=zr[:], start=True, stop=False)
                groups = [(wn, blk, t1[blk, wn]) for wn in range(NW)
                          if r1[bg][wn][1] > r1[bg][wn][0]
                          for blk in _bg_blocks(bg)]
                last_info = last_tile_of_group(groups)
                for wn in range(NW):
                    agg_group(st1, IDX1, SIN1, ps, F1, X[WB1[wn]:WB1[wn + 1], :],
                              bg, wn, last_info, queue=(bg + wn) % 4,
                              fp32_src=False, gtag="g1")
                acc1 = apool.tile([128, 512], f32, tag="acc1")
                nc.vector.tensor_copy(out=acc1[:], in_=ps[:])
                dps = pdense.tile([128, 512], f32, tag="dps")
                nc.tensor.matmul(out=dps[:], lhsT=w1t[:], rhs=acc1[:], start=True, stop=True)
                y1 = apool.tile([128, 512], f32, tag="y1")
                nc.scalar.activation(out=y1[:], in_=dps[:], func=Act.Relu, bias=b1t[:])
                for k in range(nch):
                    blk = 4 * bg + k
                    wp = pw2.tile([128, F2], f32, tag="wp")
                    nc.tensor.matmul(out=wp[:], lhsT=y1[:, k * 128:(k + 1) * 128],
                                     rhs=w2t[:], start=True, stop=True)
                    h2 = apool.tile([128, F2], f32, tag="h2")
                    nc.vector.tensor_copy(out=h2[:], in_=wp[:])
                    nc.scalar.activation(out=h2bf[:, blk, :], in_=wp[:], func=Act.Copy)
                    r0 = 512 * bg + 128 * k
                    r = r0
                    while r < r0 + 128:
                        j = r // STRIPE
                        take = min(STRIPE * (j + 1) - r, r0 + 128 - r)
                        nc.sync.dma_start(
                            out=ag_in[j][r - STRIPE * j: r - STRIPE * j + take, :],
                            in_=h2[r - r0: r - r0 + take, :])
                        r += take
                for j in ag_fire.get(bg, []):
                    nc.gpsimd.collective_compute(
                        "AllGather", Alu.bypass,
                        replica_groups=[list(range(CORES))],
                        ins=[ag_in[j][:]],
                        outs=[out_w[j][:]],
                    )

            # ---------------- layer 2 ----------------
            r2, t2 = st2["ranges"], st2["tmax"]
            first_flush = [None] * NBG     # first non-empty wn per bg
            for bg in range(NBG):
                for wn in range(NW):
                    if r2[bg][wn][1] > r2[bg][wn][0]:
                        first_flush[bg] = wn
                        break
            for wn in range(NW):
                for bg in range(NBG):
                    a, b = r2[bg][wn]
                    if a == b:
                        continue
                    width = 128 * len(list(_bg_blocks(bg)))
                    ps2 = pagg2.tile([F2, 512], f32, tag="aggps2")
                    groups = [(wn, blk, t2[blk, wn]) for blk in _bg_blocks(bg)]
                    last_info = last_tile_of_group(groups)
                    nc.tensor.matmul(out=ps2[:], lhsT=zl2[:], rhs=zr[:],
                                     start=True, stop=False)
                    if L2SELF_DENSE and first_flush[bg] == wn:
                        # self-loop diagonal: snorm_d * h2[d], via identity matmul
                        # (start=False: one psum group per bank, opened above)
                        for blk in _bg_blocks(bg):
                            off = (blk - 4 * bg) * 128
                            hs = spool.tile([128, F2], bf16, tag="hs")
                            nc.vector.tensor_tensor(
                                out=hs[:], in0=h2bf[:, blk, :],
                                in1=snormt[:, blk:blk + 1].to_broadcast([128, F2]),
                                op=Alu.mult)
                            nc.tensor.matmul(out=ps2[:, off:off + 128], lhsT=hs[:],
                                             rhs=identt[:], start=False, stop=False)
                    agg_group(st2, IDX2, SIN2, ps2, F2, out_w[wn][:],
                              bg, wn, last_info, queue=(bg + wn) % 4,
                              fp32_src=True, gtag="g2")
                    sl = acc2[:, 512 * bg: 512 * bg + width]
                    if first_flush[bg] == wn:
                        nc.vector.tensor_copy(out=sl, in_=ps2[:, :width])
                    else:
                        nc.vector.tensor_tensor(out=sl, in0=sl, in1=ps2[:, :width],
                                                op=Alu.add)

            # ---------------- epilogue ----------------
            for bg in range(NBG):
                width = 128 * len(list(_bg_blocks(bg)))
                ot = apool.tile([F2, 512], f32, tag="ot")
                nc.scalar.activation(out=ot[:, :width], in_=acc2[:, 512 * bg:512 * bg + width],
                                     func=Act.Relu, bias=b2t[:])
                nc.sync.dma_start(out=OUT[:, 512 * bg:512 * bg + width], in_=ot[:, :width])

    nc.compile()
    return nc


def kernel(x, edge_index, edge_weights, W1, b1, W2, b2, trace=False):
    import ml_dtypes
    from concourse.bass_utils import run_bass_kernel_spmd

    x = np.asarray(x, dtype=np.float32)
    W1 = np.ascontiguousarray(np.asarray(W1, dtype=np.float32))
    W2 = np.ascontiguousarray(np.asarray(W2, dtype=np.float32))
    b1 = np.asarray(b1, dtype=np.float32)
    b2 = np.asarray(b2, dtype=np.float32)

    idx1, s1, st1, idx2, s2, st2, snorm = _prep(edge_index, edge_weights)
    nc = _build_program(st1, st2)

    xpad = np.zeros((NPAD, F1), dtype=ml_dtypes.bfloat16)
    xpad[:N] = x.astype(ml_dtypes.bfloat16)
    ident = np.eye(128, dtype=np.float32).astype(ml_dtypes.bfloat16)
    in_maps = []
    for c in range(CORES):
        in_maps.append({
            "x": xpad,
            "idx1": idx1[c], "sin1": s1[c],
            "idx2": idx2[c], "sin2": s2[c],
            "snorm": snorm[c], "ident": ident,
            "w1": W1, "w2": W2,
            "b1": b1.reshape(F1, 1), "b2": b2.reshape(F2, 1),
        })

    res = run_bass_kernel_spmd(nc, in_maps, list(range(CORES)), trace=trace)
    kernel.last_result = res

    o2g = _owned_to_global()
    out_full = np.zeros((NPAD, F2), dtype=np.float32)
    for c in range(CORES):
        out_full[o2g[c]] = res.results[c]["out"].T
    return out_full[:N]
